# revision 25
# baseline (speedup 1.0000x reference)
"""AddAttention (Bahdanau additive attention) Trainium2 kernel, v2.

Math (per batch b):
    q   = query @ Wq + bq                          [D]
    k_t = value[t] @ Wk + bk                       [T, D]
    s_t = sum_d scale[d] * tanh(q[d] + k_t[d])     [T]
    a   = softmax(s masked to t < value_lens[b])
    out = sum_t a_t * value[t]                     [DV]

Distribution: pure data-parallel over batch B=32 across 8 NeuronCores
(4 batches per core, params replicated, no collectives).  Batches sorted
by value_lens; slot s on every core holds a batch of similar length so
the SPMD graph only processes ceil(max_len_in_slot/128) chunks per slot.

v2 design ("kT layout"):
  - k projection computed TRANSPOSED: kT[d, t'] = sum_j Wk[j, d]*value[t', j]
    via matmuls with Wk pieces as the stationary operand and value^T (in
    fp8e3m4 - 4 mantissa bits, rel err ~7e-3 end to end) as the moving
    operand.  d lands on partitions, so:
      * q (+bq+bk) is a per-partition BIAS of the tanh activation (free)
      * scale becomes the stationary [128,1] of a score-reduce matmul
        (kills the big DVE multiply+accum pass of v1)
  - score-reduce matmuls for the (up to) 4 pieces of a slot are packed
    into 4 concurrent PE column-groups via tile_position -> 4x faster,
    and all land in one PSUM bank at partitions {0,32,64,96}
  - scores come out t-major [1, T']; rows are packed host-side as
    t' = 512q + p*w_q + c2 so the transpose back to [128 part, chunks]
    is a DMA of 128 contiguous little runs per piece (cheap, off the
    critical path)
  - context = attn @ value also col-group packed 4x (M=1 matmuls), with
    partial rows at {0,32,64,96} summed by one selector matmul
  - q itself computed transposed on device with Wq stationary (16 tiny
    matmuls in the warmup window)
"""

import math
from contextlib import ExitStack

import ml_dtypes
import numpy as np

import concourse.bass as bass
import concourse.bacc as bacc
import concourse.tile as tile
from concourse import mybir
from concourse import bass_utils

F32 = mybir.dt.float32
BF16 = mybir.dt.bfloat16
FP8E3 = mybir.dt.float8e3
I32 = mybir.dt.int32
AF = mybir.ActivationFunctionType
ALU = mybir.AluOpType

N_CORES = 8
B, TV, DQ, DV, D = 32, 2048, 512, 512, 512
SLOTS = B // N_CORES  # 4 batches per core
P = 128
KC = D // P  # 4 contraction chunks of 128

BF16_NP = ml_dtypes.bfloat16
E3_NP = ml_dtypes.float8_e3m4


def _piece_widths(nch):
    """Slot of nch 128-row chunks -> pieces of <=4 chunks (512 t'-cols)."""
    out = []
    rem = nch
    while rem > 0:
        w = min(4, rem)
        out.append(w)
        rem -= w
    return out


def _nat_index(nch):
    """Row order of the natural-layout value pack: position p*nch + c
    (partition-major) holds packed row t' = 512q + p*w_q + c2."""
    idx = np.empty((P, nch), np.int64)
    for q, w in enumerate(_piece_widths(nch)):
        pcol = np.arange(P)[:, None] * w
        idx[:, 4 * q:4 * q + w] = 512 * q + pcol + np.arange(w)[None, :]
    return idx.reshape(-1)


def build_graph(nchunks):
    """Per-core Bass graph. nchunks[s] = 128-row chunks for slot s
    (descending; same on every core -> same SPMD graph)."""
    nchunks = tuple(int(c) for c in nchunks)
    assert len(nchunks) == SLOTS and all(1 <= c <= TV // P for c in nchunks)
    rows = [P * c for c in nchunks]
    row_off = np.cumsum([0] + rows).tolist()
    R = row_off[-1]

    nc = bacc.Bacc("TRN2", target_bir_lowering=False, debug=False,
                   enable_asserts=False)

    value_d = nc.dram_tensor("value", [R, DV], BF16, kind="ExternalInput")
    valueT_d = nc.dram_tensor("valueT", [DV, R], FP8E3, kind="ExternalInput")
    # all small params packed in ONE f32 blob (single DMA, one descriptor
    # run per partition): cols 0:16 queryT [kc,s], 16:20 scaleT, 20:24
    # (bq+bk) d-major, 24:28 lens broadcast.  Separate small loads each
    # cost ~1-6us of descriptor-bound DMA time at startup (measured).
    smalls_d = nc.dram_tensor("smalls", [P, 28], F32, kind="ExternalInput")
    # Wq and Wk stacked into one tensor -> one DMA (each serialized DMA
    # pays ~2.3us of dispatch+delay+sem fixed latency at startup)
    Wqk_d = nc.dram_tensor("Wqk", [DQ, 2, D], BF16, kind="ExternalInput")
    out_d = nc.dram_tensor("out", [SLOTS, DV], F32, kind="ExternalOutput")

    with tile.TileContext(nc) as tc, ExitStack() as ctx:
        consts = ctx.enter_context(tc.tile_pool(name="consts", bufs=1))
        vt_pool = ctx.enter_context(tc.tile_pool(name="vt", bufs=3))
        v_pool = ctx.enter_context(tc.tile_pool(name="vsb", bufs=3))
        th_pool = ctx.enter_context(tc.tile_pool(name="th", bufs=9))
        sm_pool = ctx.enter_context(tc.tile_pool(name="sm", bufs=2))
        kps_pool = ctx.enter_context(
            tc.tile_pool(name="kps", bufs=3, space=bass.MemorySpace.PSUM))
        score_pool = ctx.enter_context(
            tc.tile_pool(name="scps", bufs=2, space=bass.MemorySpace.PSUM))
        ctx_pool = ctx.enter_context(
            tc.tile_pool(name="ctxps", bufs=2, space=bass.MemorySpace.PSUM))
        misc_pool = ctx.enter_context(
            tc.tile_pool(name="mps", bufs=1, space=bass.MemorySpace.PSUM))

        # ---- constants / setup ----
        ones_row = consts.tile([1, P], BF16)
        nc.vector.memset(ones_row[:], 1.0)
        ones512 = consts.tile([1, D], BF16)
        nc.vector.memset(ones512[:], 1.0)
        ones_col_f = consts.tile([P, 1], F32)
        nc.vector.memset(ones_col_f[:], 1.0)

        # Setup params on the scalar queue: smalls blob, then Wq, then Wk.
        # The big value loads go on the sync queue, GATED behind Wk (see
        # gate tiles below): DMA engines round-robin all active transfers,
        # so letting the MB-scale value traffic start early starves the
        # param loads and stalls the PE for ~19us (measured).
        smalls = consts.tile([P, 28], F32)
        nc.scalar.dma_start(smalls[:], smalls_d.ap())

        # These are emitted BEFORE the Wqk dma_start on purpose: every
        # instruction conservatively waits on ALL DMAs emitted earlier on
        # the queue it depends on, so emitting them later would chain them
        # (and the gated VT loads) behind the 1MB Wqk transfer.
        QT_sb = consts.tile([P, KC, SLOTS], BF16)
        nc.vector.tensor_copy(QT_sb[:].rearrange("p a b -> p (a b)"),
                              smalls[:, 0:16])
        scaleT_sb = consts.tile([P, KC], BF16)
        nc.vector.tensor_copy(scaleT_sb[:], smalls[:, 16:20])
        # gate: fill every vt-pool buffer with a dummy tile whose writer
        # depends on the smalls load; the first real VT dma into each
        # buffer then waits (WAW) so it can't starve the smalls load, but
        # runs concurrently with the (bigger) Wqk transfer.
        for i in range(3):
            g = vt_pool.tile([1, 1], F32, tag="vt", name=f"gate{i}")
            nc.vector.tensor_copy(g[:], smalls[0:1, 0:1])

        Wqk_sb = consts.tile([P, KC, 2, D], BF16)
        nc.scalar.dma_start(
            Wqk_sb[:], Wqk_d.ap().rearrange("(c p) w n -> p c w n", p=P))

        # per-slot selector columns for the 4-partial-row context reduce
        sel = []
        for s in range(SLOTS):
            ng = min(4, nchunks[s])
            sl = consts.tile([P, 1], BF16, tag=f"sel{s}")
            nc.vector.memset(sl[:], 0.0)
            for g in range(ng):
                nc.vector.memset(sl[32 * g:32 * g + 1, :], 1.0)
            sel.append(sl)

        # PE warm-up: ~5 us of memset-only matmuls inside the startup DMA
        # window flips the HAM clock gate to 2.4 GHz before real work lands
        for w in range(20):
            wu = kps_pool.tile([P, D], F32, tag="kps", name=f"wu{w}")
            nc.tensor.matmul(wu[:], ones_row[:], ones512[:],
                             start=True, stop=True)

        # zero the two context PSUM banks once: unwritten partitions must
        # read as finite 0.0 for the selector-matmul reduce
        for i in range(2):
            z = ctx_pool.tile([P, 512], F32, tag="ctx4", name=f"ctxz{i}")
            nc.vector.memset(z[:], 0.0)

        # q computed TRANSPOSED: qT[d, s] = sum_k Wq[k, d] queryT[k, s],
        # Wq pieces stationary; then add (bq+bk)[d] per partition.
        qT_sb = consts.tile([P, KC, SLOTS], F32)
        for dc in range(KC):
            qt_ps = kps_pool.tile([P, SLOTS], F32, tag="kps", name=f"qt{dc}")
            for kc in range(KC):
                nc.tensor.matmul(qt_ps[:], Wqk_sb[:, kc, 0, dc * P:(dc + 1) * P],
                                 QT_sb[:, kc, :],
                                 start=(kc == 0), stop=(kc == KC - 1))
            nc.vector.tensor_scalar(qT_sb[:, dc, :], qt_ps[:],
                                    smalls[:, 20 + dc:21 + dc], None,
                                    op0=ALU.add)

        # masks: mask_s[p, 4q+c2] = (512q + p*w_q + c2 < len_s)
        mask = []
        for s in range(SLOTS):
            nch = nchunks[s]
            iota_i = consts.tile([P, nch], I32, tag=f"io{s}")
            for q, w in enumerate(_piece_widths(nch)):
                nc.gpsimd.iota(iota_i[:, 4 * q:4 * q + w], pattern=[[1, w]],
                               base=512 * q, channel_multiplier=w)
            iota_f = consts.tile([P, nch], F32, tag=f"iof{s}")
            nc.vector.tensor_copy(iota_f[:], iota_i[:])
            mk = consts.tile([P, nch], F32, tag=f"mask{s}")
            nc.vector.tensor_scalar(mk[:], iota_f[:], smalls[:, 24 + s:25 + s],
                                    None, op0=ALU.is_lt)
            mask.append(mk)

        # ---- per-slot emission, software-pipelined ----
        def emit_tail(s, score4, V_sb):
            nch = nchunks[s]
            widths = _piece_widths(nch)
            # scores [1, T'] live at partitions {0,32,64,96} of score4
            s4sb = sm_pool.tile([P, 512], F32, tag="s4sb", name=f"s4sb{s}")
            nc.vector.tensor_copy(s4sb[:], score4[:])
            scT = sm_pool.tile([P, nch], F32, tag="scT", name=f"scT{s}")
            for q, w in enumerate(widths):
                src = s4sb[32 * q:32 * q + 1, 0:P * w].rearrange(
                    "a (p c) -> a p c", p=P)
                nc.gpsimd.dma_start(scT[:, 4 * q:4 * q + w], src)
            ex = sm_pool.tile([P, nch], F32, tag="ex", name=f"ex{s}")
            nc.scalar.activation(ex[:], scT[:], AF.Exp)
            exm = sm_pool.tile([P, nch], F32, tag="exm", name=f"exm{s}")
            nc.vector.tensor_tensor(exm[:], ex[:], mask[s][:], op=ALU.mult)
            rs = sm_pool.tile([P, 1], F32, tag="rs", name=f"rs{s}")
            nc.vector.reduce_sum(rs[:], exm[:], axis=mybir.AxisListType.X)
            attn = sm_pool.tile([P, nch], BF16, tag="attn", name=f"attn{s}")
            nc.vector.tensor_copy(attn[:], exm[:])
            stot = misc_pool.tile([1, 1], F32, tag="m", name=f"stot{s}")
            nc.tensor.matmul(stot[:], rs[:], ones_col_f[:],
                             start=True, stop=True)
            rcp = sm_pool.tile([1, 1], F32, tag="rcp", name=f"rcp{s}")
            nc.vector.reciprocal(rcp[:], stot[:])

            # context: M=1 matmuls, chunk c -> column-group (c mod 4)
            ctx4 = ctx_pool.tile([P, 512], F32, tag="ctx4", name=f"ctx4_{s}")
            for g in range(min(4, nch)):
                chain = list(range(g, nch, 4))
                for i, c in enumerate(chain):
                    nc.tensor.matmul(ctx4[32 * g:32 * g + 1, :],
                                     attn[:, c:c + 1], V_sb[:, c, :],
                                     start=(i == 0), stop=(i == len(chain) - 1),
                                     tile_position=(0, 32 * g))
            c4sb = sm_pool.tile([P, 512], BF16, tag="c4sb", name=f"c4sb{s}")
            nc.vector.tensor_copy(c4sb[:], ctx4[:])
            ctxred = misc_pool.tile([1, DV], F32, tag="m", name=f"cred{s}")
            nc.tensor.matmul(ctxred[:], sel[s][:], c4sb[:],
                             start=True, stop=True)
            orow = sm_pool.tile([1, DV], F32, tag="orow", name=f"orow{s}")
            nc.vector.tensor_scalar(orow[:], ctxred[:], rcp[:], None,
                                    op0=ALU.mult)
            nc.scalar.dma_start(out_d[s:s + 1, :], orow[:])

        pending = None
        for s in range(SLOTS):
            nch = nchunks[s]
            widths = _piece_widths(nch)
            VT_sb = vt_pool.tile([P, KC, rows[s]], FP8E3, tag="vt",
                                 name=f"vt{s}")
            nc.sync.dma_start(
                VT_sb[:],
                valueT_d[:, row_off[s]:row_off[s + 1]].rearrange(
                    "(j p) t -> p j t", p=P))
            score4 = score_pool.tile([P, 512], F32, tag="s4", name=f"s4{s}")

            ths = {}

            def kproj_piece(q, w):
                W = P * w
                c0 = 512 * q
                for dc in range(KC):
                    kps = kps_pool.tile([P, W], F32, tag="kps",
                                        name=f"k{s}_{q}_{dc}")
                    for j in range(KC):
                        nc.tensor.matmul(kps[:],
                                         Wqk_sb[:, j, 1, dc * P:(dc + 1) * P],
                                         VT_sb[:, j, c0:c0 + W],
                                         start=(j == 0), stop=(j == KC - 1))
                    th = th_pool.tile([P, W], BF16, tag="th",
                                      name=f"th{s}_{q}_{dc}")
                    nc.scalar.activation(th[:], kps[:], AF.Tanh,
                                         bias=qT_sb[:, dc, s:s + 1])
                    ths[(q, dc)] = th

            def score_piece(q, w):
                W = P * w
                for dc in range(KC):
                    nc.tensor.matmul(score4[32 * q:32 * q + 1, 0:W],
                                     scaleT_sb[:, dc:dc + 1],
                                     ths.pop((q, dc))[:],
                                     start=(dc == 0), stop=(dc == KC - 1),
                                     tile_position=(0, 32 * q))

            prev_piece = None
            tail_done = pending is None
            for q, w in enumerate(widths):
                kproj_piece(q, w)
                if not tail_done:
                    emit_tail(*pending)
                    tail_done = True
                if prev_piece is not None:
                    score_piece(*prev_piece)
                prev_piece = (q, w)
            score_piece(*prev_piece)
            # V load emitted LAST: every instruction conservatively waits
            # on all earlier DMAs of its queue, so emitting this before the
            # kproj matmuls would make them wait for data only the (much
            # later) context matmuls need.  Queue position still keeps the
            # transfer right behind VT(s) on the sync queue.
            V_sb = v_pool.tile([P, nch, DV], BF16, tag="vsb", name=f"vsb{s}")
            nc.sync.dma_start(
                V_sb[:],
                value_d[row_off[s]:row_off[s + 1], :].rearrange(
                    "(p c) d -> p c d", p=P))
            pending = (s, score4, V_sb)
        emit_tail(*pending)

    nc.compile()
    return nc


_graph_cache = {}

# test-harness knobs (the grading path leaves these at defaults)
TRACE = False
TRACE_KWARGS = {}
LAST_RESULTS = None


def _get_graph(nchunks):
    key = tuple(nchunks)
    if key not in _graph_cache:
        _graph_cache[key] = build_graph(key)
    return _graph_cache[key]


def plan(value_lens):
    """Sort batches by length desc; rank r -> core r%8, slot r//8."""
    lens = np.asarray(value_lens, np.int64)
    order = np.argsort(-lens, kind="stable")
    nchunks = tuple(
        int(math.ceil(max(1, int(lens[order[s * N_CORES:(s + 1) * N_CORES]].max())) / P))
        for s in range(SLOTS))
    return order, nchunks


def prepare(query, value, value_lens, Wq, bq, Wk, bk, scale):
    query = np.ascontiguousarray(np.asarray(query, np.float32))
    value = np.ascontiguousarray(np.asarray(value, np.float32))
    lens = np.ascontiguousarray(np.asarray(value_lens, np.int32))
    Wq = np.ascontiguousarray(np.asarray(Wq, np.float32))
    bq = np.ascontiguousarray(np.asarray(bq, np.float32))
    Wk = np.ascontiguousarray(np.asarray(Wk, np.float32))
    bk = np.ascontiguousarray(np.asarray(bk, np.float32))
    scale = np.ascontiguousarray(np.asarray(scale, np.float32))

    order, nchunks = plan(lens)
    nc = _get_graph(nchunks)

    nat_idx = [_nat_index(nchunks[s]) for s in range(SLOTS)]
    scaleT = scale.reshape(KC, P).T.astype(np.float32)
    bqbkT = (bq + bk).reshape(KC, P).T.astype(np.float32)
    Wqk_b = np.ascontiguousarray(
        np.stack([Wq, Wk], axis=1).astype(BF16_NP))  # [DQ, 2, D]

    in_maps = []
    for c in range(N_CORES):
        bidx = [int(order[s * N_CORES + c]) for s in range(SLOTS)]
        vparts, vtparts = [], []
        for s in range(SLOTS):
            T = nchunks[s] * P
            vp = value[bidx[s], :T, :]  # [T, DV] f32 (T <= TV always)
            vtparts.append(vp.T.astype(E3_NP))
            vparts.append(vp[nat_idx[s]].astype(BF16_NP))
        qt = query[bidx]  # [SLOTS, DQ] f32
        qt_cols = qt.T.reshape(KC, P, SLOTS).transpose(1, 0, 2).reshape(P, 16)
        lens_bc = np.broadcast_to(lens[bidx].astype(np.float32)[None, :],
                                  (P, SLOTS))
        smalls = np.concatenate(
            [qt_cols.astype(np.float32), scaleT, bqbkT, lens_bc], axis=1)
        in_maps.append({
            "value": np.ascontiguousarray(np.concatenate(vparts, axis=0)),
            "valueT": np.ascontiguousarray(np.concatenate(vtparts, axis=1)),
            "smalls": np.ascontiguousarray(smalls.astype(np.float32)),
            "Wqk": Wqk_b,
        })
    return nc, in_maps, order, nchunks


def kernel(query, value, value_lens, Wq, bq, Wk, bk, scale):
    nc, in_maps, order, _ = prepare(query, value, value_lens,
                                    Wq, bq, Wk, bk, scale)

    res = bass_utils.run_bass_kernel_spmd(
        nc, in_maps, core_ids=list(range(N_CORES)), trace=TRACE,
        **TRACE_KWARGS)
    global LAST_RESULTS
    LAST_RESULTS = res

    out = np.zeros((B, 1, DV), np.float32)
    for c in range(N_CORES):
        o = res.results[c]["out"]
        for s in range(SLOTS):
            out[int(order[s * N_CORES + c]), 0, :] = o[s]
    return out


# revision 26
# speedup vs baseline: 1.0700x; 1.0700x over previous
"""AddAttention (Bahdanau additive attention) Trainium2 kernel, v2.

Math (per batch b):
    q   = query @ Wq + bq                          [D]
    k_t = value[t] @ Wk + bk                       [T, D]
    s_t = sum_d scale[d] * tanh(q[d] + k_t[d])     [T]
    a   = softmax(s masked to t < value_lens[b])
    out = sum_t a_t * value[t]                     [DV]

Distribution: pure data-parallel over batch B=32 across 8 NeuronCores
(4 batches per core, params replicated, no collectives).  Batches sorted
by value_lens; slot s on every core holds a batch of similar length so
the SPMD graph only processes ceil(max_len_in_slot/128) chunks per slot.

v2 design ("kT layout"):
  - k projection computed TRANSPOSED: kT[d, t'] = sum_j Wk[j, d]*value[t', j]
    via matmuls with Wk pieces as the stationary operand and value^T (in
    fp8e3m4 - 4 mantissa bits, rel err ~7e-3 end to end) as the moving
    operand.  d lands on partitions, so:
      * q (+bq+bk) is a per-partition BIAS of the tanh activation (free)
      * scale becomes the stationary [128,1] of a score-reduce matmul
        (kills the big DVE multiply+accum pass of v1)
  - score-reduce matmuls for the (up to) 4 pieces of a slot are packed
    into 4 concurrent PE column-groups via tile_position -> 4x faster,
    and all land in one PSUM bank at partitions {0,32,64,96}
  - scores come out t-major [1, T']; rows are packed host-side as
    t' = 512q + p*w_q + c2 so the transpose back to [128 part, chunks]
    is a DMA of 128 contiguous little runs per piece (cheap, off the
    critical path)
  - context = attn @ value also col-group packed 4x (M=1 matmuls), with
    partial rows at {0,32,64,96} summed by one selector matmul
  - q itself computed transposed on device with Wq stationary (16 tiny
    matmuls in the warmup window)
"""

import math
from contextlib import ExitStack

import ml_dtypes
import numpy as np

import concourse.bass as bass
import concourse.bacc as bacc
import concourse.tile as tile
from concourse import mybir
from concourse import bass_utils

F32 = mybir.dt.float32
BF16 = mybir.dt.bfloat16
FP8E3 = mybir.dt.float8e3
I32 = mybir.dt.int32
AF = mybir.ActivationFunctionType
ALU = mybir.AluOpType

N_CORES = 8
B, TV, DQ, DV, D = 32, 2048, 512, 512, 512
SLOTS = B // N_CORES  # 4 batches per core
P = 128
KC = D // P  # 4 contraction chunks of 128

BF16_NP = ml_dtypes.bfloat16
E3_NP = ml_dtypes.float8_e3m4


def _piece_widths(nch):
    """Slot of nch 128-row chunks -> pieces of <=4 chunks (512 t'-cols)."""
    out = []
    rem = nch
    while rem > 0:
        w = min(4, rem)
        out.append(w)
        rem -= w
    return out


def _nat_index(nch):
    """Row order of the natural-layout value pack: position p*nch + c
    (partition-major) holds packed row t' = 512q + p*w_q + c2."""
    idx = np.empty((P, nch), np.int64)
    for q, w in enumerate(_piece_widths(nch)):
        pcol = np.arange(P)[:, None] * w
        idx[:, 4 * q:4 * q + w] = 512 * q + pcol + np.arange(w)[None, :]
    return idx.reshape(-1)


def build_graph(nchunks):
    """Per-core Bass graph. nchunks[s] = 128-row chunks for slot s
    (descending; same on every core -> same SPMD graph)."""
    nchunks = tuple(int(c) for c in nchunks)
    assert len(nchunks) == SLOTS and all(1 <= c <= TV // P for c in nchunks)
    rows = [P * c for c in nchunks]
    row_off = np.cumsum([0] + rows).tolist()
    R = row_off[-1]

    nc = bacc.Bacc("TRN2", target_bir_lowering=False, debug=False,
                   enable_asserts=False)

    value_d = nc.dram_tensor("value", [R, DV], BF16, kind="ExternalInput")
    valueT_d = nc.dram_tensor("valueT", [DV, R], FP8E3, kind="ExternalInput")
    # all small params packed in ONE f32 blob (single DMA, one descriptor
    # run per partition): cols 0:16 queryT [kc,s], 16:20 scaleT, 20:24
    # (bq+bk) d-major, 24:28 lens broadcast.  Separate small loads each
    # cost ~1-6us of descriptor-bound DMA time at startup (measured).
    smalls_d = nc.dram_tensor("smalls", [P, 28], F32, kind="ExternalInput")
    # Wq and Wk stacked into one tensor -> one DMA (each serialized DMA
    # pays ~2.3us of dispatch+delay+sem fixed latency at startup)
    Wqk_d = nc.dram_tensor("Wqk", [DQ, 2, D], BF16, kind="ExternalInput")
    out_d = nc.dram_tensor("out", [SLOTS, DV], F32, kind="ExternalOutput")

    with tile.TileContext(nc) as tc, ExitStack() as ctx:
        consts = ctx.enter_context(tc.tile_pool(name="consts", bufs=1))
        vt_pool = ctx.enter_context(tc.tile_pool(name="vt", bufs=3))
        v_pool = ctx.enter_context(tc.tile_pool(name="vsb", bufs=3))
        th_pool = ctx.enter_context(tc.tile_pool(name="th", bufs=9))
        sm_pool = ctx.enter_context(tc.tile_pool(name="sm", bufs=2))
        kps_pool = ctx.enter_context(
            tc.tile_pool(name="kps", bufs=3, space=bass.MemorySpace.PSUM))
        score_pool = ctx.enter_context(
            tc.tile_pool(name="scps", bufs=2, space=bass.MemorySpace.PSUM))
        ctx_pool = ctx.enter_context(
            tc.tile_pool(name="ctxps", bufs=2, space=bass.MemorySpace.PSUM))
        misc_pool = ctx.enter_context(
            tc.tile_pool(name="mps", bufs=1, space=bass.MemorySpace.PSUM))

        # ---- constants / setup ----
        ones_row = consts.tile([1, P], BF16)
        nc.vector.memset(ones_row[:], 1.0)
        ones512 = consts.tile([1, D], BF16)
        nc.vector.memset(ones512[:], 1.0)
        ones_col_f = consts.tile([P, 1], F32)
        nc.vector.memset(ones_col_f[:], 1.0)

        # Setup params on the scalar queue: smalls blob, then Wq, then Wk.
        # The big value loads go on the sync queue, GATED behind Wk (see
        # gate tiles below): DMA engines round-robin all active transfers,
        # so letting the MB-scale value traffic start early starves the
        # param loads and stalls the PE for ~19us (measured).
        smalls = consts.tile([P, 28], F32)
        nc.scalar.dma_start(smalls[:], smalls_d.ap())

        # These are emitted BEFORE the Wqk dma_start on purpose: every
        # instruction conservatively waits on ALL DMAs emitted earlier on
        # the queue it depends on, so emitting them later would chain them
        # (and the gated VT loads) behind the 1MB Wqk transfer.
        QT_sb = consts.tile([P, KC, SLOTS], BF16)
        nc.vector.tensor_copy(QT_sb[:].rearrange("p a b -> p (a b)"),
                              smalls[:, 0:16])
        scaleT_sb = consts.tile([P, KC], BF16)
        nc.vector.tensor_copy(scaleT_sb[:], smalls[:, 16:20])
        Wqk_sb = consts.tile([P, KC, 2, D], BF16)
        nc.scalar.dma_start(
            Wqk_sb[:], Wqk_d.ap().rearrange("(c p) w n -> p c w n", p=P))

        # per-slot selector columns for the 4-partial-row context reduce
        sel = []
        for s in range(SLOTS):
            ng = min(4, nchunks[s])
            sl = consts.tile([P, 1], BF16, tag=f"sel{s}")
            nc.vector.memset(sl[:], 0.0)
            for g in range(ng):
                nc.vector.memset(sl[32 * g:32 * g + 1, :], 1.0)
            sel.append(sl)

        # PE warm-up: ~5 us of memset-only matmuls inside the startup DMA
        # window flips the HAM clock gate to 2.4 GHz before real work lands
        for w in range(20):
            wu = kps_pool.tile([P, D], F32, tag="kps", name=f"wu{w}")
            nc.tensor.matmul(wu[:], ones_row[:], ones512[:],
                             start=True, stop=True)

        # zero the two context PSUM banks once: unwritten partitions must
        # read as finite 0.0 for the selector-matmul reduce
        for i in range(2):
            z = ctx_pool.tile([P, 512], F32, tag="ctx4", name=f"ctxz{i}")
            nc.vector.memset(z[:], 0.0)

        # q computed TRANSPOSED: qT[d, s] = sum_k Wq[k, d] queryT[k, s],
        # Wq pieces stationary; then add (bq+bk)[d] per partition.
        qT_sb = consts.tile([P, KC, SLOTS], F32)
        for dc in range(KC):
            qt_ps = misc_pool.tile([P, SLOTS], F32, tag="m", name=f"qt{dc}")
            for kc in range(KC):
                nc.tensor.matmul(qt_ps[:], Wqk_sb[:, kc, 0, dc * P:(dc + 1) * P],
                                 QT_sb[:, kc, :],
                                 start=(kc == 0), stop=(kc == KC - 1))
            nc.vector.tensor_scalar(qT_sb[:, dc, :], qt_ps[:],
                                    smalls[:, 20 + dc:21 + dc], None,
                                    op0=ALU.add)

        # masks: mask_s[p, 4q+c2] = (512q + p*w_q + c2 < len_s)
        mask = []
        for s in range(SLOTS):
            nch = nchunks[s]
            iota_i = consts.tile([P, nch], I32, tag=f"io{s}")
            for q, w in enumerate(_piece_widths(nch)):
                nc.gpsimd.iota(iota_i[:, 4 * q:4 * q + w], pattern=[[1, w]],
                               base=512 * q, channel_multiplier=w)
            iota_f = consts.tile([P, nch], F32, tag=f"iof{s}")
            nc.vector.tensor_copy(iota_f[:], iota_i[:])
            mk = consts.tile([P, nch], F32, tag=f"mask{s}")
            nc.vector.tensor_scalar(mk[:], iota_f[:], smalls[:, 24 + s:25 + s],
                                    None, op0=ALU.is_lt)
            mask.append(mk)

        # ---- per-slot emission, software-pipelined ----
        def emit_tail(s, score4, V_sb):
            nch = nchunks[s]
            widths = _piece_widths(nch)
            # scores [1, T'] live at partitions {0,32,64,96} of score4
            s4sb = sm_pool.tile([P, 512], F32, tag="s4sb", name=f"s4sb{s}")
            nc.vector.tensor_copy(s4sb[:], score4[:])
            scT = sm_pool.tile([P, nch], F32, tag="scT", name=f"scT{s}")
            for q, w in enumerate(widths):
                src = s4sb[32 * q:32 * q + 1, 0:P * w].rearrange(
                    "a (p c) -> a p c", p=P)
                nc.gpsimd.dma_start(scT[:, 4 * q:4 * q + w], src)
            ex = sm_pool.tile([P, nch], F32, tag="ex", name=f"ex{s}")
            nc.scalar.activation(ex[:], scT[:], AF.Exp)
            exm = sm_pool.tile([P, nch], F32, tag="exm", name=f"exm{s}")
            nc.vector.tensor_tensor(exm[:], ex[:], mask[s][:], op=ALU.mult)
            rs = sm_pool.tile([P, 1], F32, tag="rs", name=f"rs{s}")
            nc.vector.reduce_sum(rs[:], exm[:], axis=mybir.AxisListType.X)
            attn = sm_pool.tile([P, nch], BF16, tag="attn", name=f"attn{s}")
            nc.vector.tensor_copy(attn[:], exm[:])
            stot = misc_pool.tile([1, 1], F32, tag="m", name=f"stot{s}")
            nc.tensor.matmul(stot[:], rs[:], ones_col_f[:],
                             start=True, stop=True)
            rcp = sm_pool.tile([1, 1], F32, tag="rcp", name=f"rcp{s}")
            nc.vector.reciprocal(rcp[:], stot[:])

            # context: M=1 matmuls, chunk c -> column-group (c mod 4)
            ctx4 = ctx_pool.tile([P, 512], F32, tag="ctx4", name=f"ctx4_{s}")
            for g in range(min(4, nch)):
                chain = list(range(g, nch, 4))
                for i, c in enumerate(chain):
                    nc.tensor.matmul(ctx4[32 * g:32 * g + 1, :],
                                     attn[:, c:c + 1], V_sb[:, c, :],
                                     start=(i == 0), stop=(i == len(chain) - 1),
                                     tile_position=(0, 32 * g))
            c4sb = sm_pool.tile([P, 512], BF16, tag="c4sb", name=f"c4sb{s}")
            nc.vector.tensor_copy(c4sb[:], ctx4[:])
            ctxred = misc_pool.tile([1, DV], F32, tag="m", name=f"cred{s}")
            nc.tensor.matmul(ctxred[:], sel[s][:], c4sb[:],
                             start=True, stop=True)
            orow = sm_pool.tile([1, DV], F32, tag="orow", name=f"orow{s}")
            nc.vector.tensor_scalar(orow[:], ctxred[:], rcp[:], None,
                                    op0=ALU.mult)
            nc.gpsimd.dma_start(out_d[s:s + 1, :], orow[:])

        pending = None
        for s in range(SLOTS):
            nch = nchunks[s]
            widths = _piece_widths(nch)
            VT_sb = vt_pool.tile([P, KC, rows[s]], FP8E3, tag="vt",
                                 name=f"vt{s}")
            nc.scalar.dma_start(
                VT_sb[:],
                valueT_d[:, row_off[s]:row_off[s + 1]].rearrange(
                    "(j p) t -> p j t", p=P))
            score4 = score_pool.tile([P, 512], F32, tag="s4", name=f"s4{s}")

            ths = {}

            def kproj_piece(q, w):
                W = P * w
                c0 = 512 * q
                for dc in range(KC):
                    kps = kps_pool.tile([P, W], F32, tag="kps",
                                        name=f"k{s}_{q}_{dc}")
                    for j in range(KC):
                        nc.tensor.matmul(kps[:],
                                         Wqk_sb[:, j, 1, dc * P:(dc + 1) * P],
                                         VT_sb[:, j, c0:c0 + W],
                                         start=(j == 0), stop=(j == KC - 1))
                    th = th_pool.tile([P, W], BF16, tag="th",
                                      name=f"th{s}_{q}_{dc}")
                    nc.scalar.activation(th[:], kps[:], AF.Tanh,
                                         bias=qT_sb[:, dc, s:s + 1])
                    ths[(q, dc)] = th

            def score_piece(q, w):
                W = P * w
                for dc in range(KC):
                    nc.tensor.matmul(score4[32 * q:32 * q + 1, 0:W],
                                     scaleT_sb[:, dc:dc + 1],
                                     ths.pop((q, dc))[:],
                                     start=(dc == 0), stop=(dc == KC - 1),
                                     tile_position=(0, 32 * q))

            prev_piece = None
            tail_done = pending is None
            for q, w in enumerate(widths):
                kproj_piece(q, w)
                if not tail_done:
                    emit_tail(*pending)
                    tail_done = True
                if prev_piece is not None:
                    score_piece(*prev_piece)
                prev_piece = (q, w)
            score_piece(*prev_piece)
            # V load emitted LAST: every instruction conservatively waits
            # on all earlier DMAs of its queue, so emitting this before the
            # kproj matmuls would make them wait for data only the (much
            # later) context matmuls need.  Queue position still keeps the
            # transfer right behind VT(s) on the sync queue.
            V_sb = v_pool.tile([P, nch, DV], BF16, tag="vsb", name=f"vsb{s}")
            nc.scalar.dma_start(
                V_sb[:],
                value_d[row_off[s]:row_off[s + 1], :].rearrange(
                    "(p c) d -> p c d", p=P))
            pending = (s, score4, V_sb)
        emit_tail(*pending)

    nc.compile()
    return nc


_graph_cache = {}

# test-harness knobs (the grading path leaves these at defaults)
TRACE = False
TRACE_KWARGS = {}
LAST_RESULTS = None


def _get_graph(nchunks):
    key = tuple(nchunks)
    if key not in _graph_cache:
        _graph_cache[key] = build_graph(key)
    return _graph_cache[key]


def plan(value_lens):
    """Sort batches by length desc; rank r -> core r%8, slot r//8."""
    lens = np.asarray(value_lens, np.int64)
    order = np.argsort(-lens, kind="stable")
    nchunks = tuple(
        int(math.ceil(max(1, int(lens[order[s * N_CORES:(s + 1) * N_CORES]].max())) / P))
        for s in range(SLOTS))
    return order, nchunks


def prepare(query, value, value_lens, Wq, bq, Wk, bk, scale):
    query = np.ascontiguousarray(np.asarray(query, np.float32))
    value = np.ascontiguousarray(np.asarray(value, np.float32))
    lens = np.ascontiguousarray(np.asarray(value_lens, np.int32))
    Wq = np.ascontiguousarray(np.asarray(Wq, np.float32))
    bq = np.ascontiguousarray(np.asarray(bq, np.float32))
    Wk = np.ascontiguousarray(np.asarray(Wk, np.float32))
    bk = np.ascontiguousarray(np.asarray(bk, np.float32))
    scale = np.ascontiguousarray(np.asarray(scale, np.float32))

    order, nchunks = plan(lens)
    nc = _get_graph(nchunks)

    nat_idx = [_nat_index(nchunks[s]) for s in range(SLOTS)]
    scaleT = scale.reshape(KC, P).T.astype(np.float32)
    bqbkT = (bq + bk).reshape(KC, P).T.astype(np.float32)
    Wqk_b = np.ascontiguousarray(
        np.stack([Wq, Wk], axis=1).astype(BF16_NP))  # [DQ, 2, D]

    in_maps = []
    for c in range(N_CORES):
        bidx = [int(order[s * N_CORES + c]) for s in range(SLOTS)]
        vparts, vtparts = [], []
        for s in range(SLOTS):
            T = nchunks[s] * P
            vp = value[bidx[s], :T, :]  # [T, DV] f32 (T <= TV always)
            vtparts.append(vp.T.astype(E3_NP))
            vparts.append(vp[nat_idx[s]].astype(BF16_NP))
        qt = query[bidx]  # [SLOTS, DQ] f32
        qt_cols = qt.T.reshape(KC, P, SLOTS).transpose(1, 0, 2).reshape(P, 16)
        lens_bc = np.broadcast_to(lens[bidx].astype(np.float32)[None, :],
                                  (P, SLOTS))
        smalls = np.concatenate(
            [qt_cols.astype(np.float32), scaleT, bqbkT, lens_bc], axis=1)
        in_maps.append({
            "value": np.ascontiguousarray(np.concatenate(vparts, axis=0)),
            "valueT": np.ascontiguousarray(np.concatenate(vtparts, axis=1)),
            "smalls": np.ascontiguousarray(smalls.astype(np.float32)),
            "Wqk": Wqk_b,
        })
    return nc, in_maps, order, nchunks


def kernel(query, value, value_lens, Wq, bq, Wk, bk, scale):
    nc, in_maps, order, _ = prepare(query, value, value_lens,
                                    Wq, bq, Wk, bk, scale)

    res = bass_utils.run_bass_kernel_spmd(
        nc, in_maps, core_ids=list(range(N_CORES)), trace=TRACE,
        **TRACE_KWARGS)
    global LAST_RESULTS
    LAST_RESULTS = res

    out = np.zeros((B, 1, DV), np.float32)
    for c in range(N_CORES):
        o = res.results[c]["out"]
        for s in range(SLOTS):
            out[int(order[s * N_CORES + c]), 0, :] = o[s]
    return out
